# revision 8
# baseline (speedup 1.0000x reference)
"""Trainium2 Bass kernel for a 50-step autoregressive MLP rollout.

reference semantics (per batch row b):
    state = x[b, 0, 2:9]                       # 7 state vars
    for t in range(50):
        u = x[b, t, 0:2]                       # 2 controls
        h1 = tanh([u, state] @ W1 + b1)        # [9] -> [256]
        h2 = tanh(h1 @ W2 + b2)                # [256] -> [256]
        d  = h2 @ W3 + b3                      # [256] -> [7]
        state = state + 0.02 * d
        out[b, t] = state
    (b1/b2/b3 are zeros per the spec and asserted so.)

Data parallel over batch across 8 NeuronCores (4096 rows each), feature-major
on-chip layout ([feature, batch]).  The per-step engine budget is dominated by
the ACT engine (two tanh layers over 256x4096 elems = 16 activation ops of
[128,1024]); so v2 makes three structural changes vs the f32r baseline:

1. mm2 (256x256, the big matmul) runs as fp8-e4m3 DoubleRow: stationary holds
   both K-planes ([128,2,128]) and the moving h1 tile supplies 2 fp8 values
   per partition, costing 0.5 cycles/row -- 4x less PE time than f32r.
   W2 is pre-scaled by 64 host-side (fp8 has no subnormal headroom at 0.1
   scale); the 1/64 descale folds into the next activation's scale operand.
   h1 itself is stored fp8 (tanh output in [-1,1]); mm1 and mm3 stay f32r so
   the state recurrence keeps high precision (empirically rel err ~1.3e-2
   end to end, within the 2e-2 gate).
2. State updates (s + PSUM) move from DVE to the otherwise-idle Pool engine
   via scalar_tensor_tensor, freeing DVE.
3. A tunable subset of the 8 per-step h2-tanh tiles is computed on DVE
   instead of ACT, via a clamped Pade(5,4) rational tanh:
       tanh(x) ~= x(t+p1)(t+p2) / (15(t+q1)(t+q2)),  t = clip(x)^2
   evaluated as two custom DVE ops (numerator / denominator, 7-8 ALU stages
   each, constants passed per-instruction so the same ops serve any input
   scale), the production RECIPROCAL_APPROX_FAST, and one fused
   scalar_tensor_tensor (u*K)*r.  Max approx error ~1.5e-3, negligible next
   to fp8 quantization noise.
"""

import numpy as np

B_TOTAL = 32768
N_CORES = 8
B_CORE = B_TOTAL // N_CORES          # 4096
H = 50
F = 9
NCTRL = 2
NST = 7
HID = 256
DT = 0.02
NTILE = 512
W2SCALE = 64.0

# Pade(5,4) tanh: num roots of t^2+105t+945, den/15 roots of t^2+28t+63
TP1, TP2 = 9.941048, 95.058952
TQ1, TQ2 = 2.467508, 25.532492
TCLAMP = 3.65

_CACHE = {}
_TANH_OPS = None


def _get_tanh_ops():
    """Register (idempotently) two custom DVE ops for the clamped Pade tanh:
    TANH_PADE_NUM: out = xc*(xc^2+s0)*(xc^2+s1), xc = clip(in0, -imm2, imm2)
    TANH_PADE_DEN: out = (xc^2+s0)*(xc^2+s1)
    Constants ride the instruction (s0/s1/imm2), so one table row serves all
    input scales."""
    global _TANH_OPS
    if _TANH_OPS is not None:
        return _TANH_OPS
    import numpy as _np
    from concourse import dve_ops as dops
    from concourse.dve_spec import C0, C1, C2, Spec, Src0, Zero, lower, maxx, minn, sq
    from concourse.dve_uop import DveOpSpec

    if "TANH_PADE_NUM" in dops._SUB_OPCODE_FOR_NAME:
        by_name = {op.name: op for op in dops.OPS}
        _TANH_OPS = (by_name["TANH_PADE_NUM"], by_name["TANH_PADE_DEN"])
        return _TANH_OPS

    def make(name, with_x, row):
        # minn first so the hoisted (Zero - C2) latch is read at stage >= 1
        xc = maxx(minn(Src0, C2), Zero - C2)
        t = sq(xc)
        body = (t + C0) * (t + C1)
        if with_x:
            body = body * xc

        def ref(in0, in1, s0, s1, imm2, _wx=with_x):
            x = _np.clip(in0.astype(_np.float32), -imm2, imm2)
            tt = x * x
            r = (tt + _np.float32(s0)) * (tt + _np.float32(s1))
            return (r * x if _wx else r).astype(_np.float32)

        spec = Spec(body=body, reference=ref)
        dops._SUB_OPCODE_FOR_NAME[name] = row
        shas = {}
        for ver in ("v3", "v4"):
            try:
                uops = lower(spec, ver=ver)
                shas[ver] = DveOpSpec(name=name, opcode=row, uops=uops,
                                      rd1_en=False).sha(ver)
            except Exception:
                pass
        op = dops.DveOp(name, spec, subdim=False, uops_sha=shas)
        dops.OPS.append(op)
        dops.CUSTOM_DVE_SPECS[name] = spec
        return op

    num = make("TANH_PADE_NUM", True, 17)
    den = make("TANH_PADE_DEN", False, 18)
    _TANH_OPS = (num, den)
    return _TANH_OPS


def _build(b_core=B_CORE, horizon=H, psa_bufs=1, psb_bufs=2, pd_tag="own", pt_tag="sm",
           chunks=(18, 18, 10, 4), spread=1, reps=1,
           off_tiles=(2, 6), mul_eng="gpsimd", state_eng="vector",
           epi_eng="vector"):
    import concourse.bacc as bacc
    import concourse.mybir as mybir
    import concourse.tile as tile

    f32 = mybir.dt.float32
    f32r = mybir.dt.float32r
    fp8 = mybir.dt.float8e4
    Tanh = mybir.ActivationFunctionType.Tanh
    Alu = mybir.AluOpType
    DR = mybir.MatmulPerfMode.DoubleRow

    tanh_num_op, tanh_den_op = _get_tanh_ops() if off_tiles else (None, None)

    # z-domain (PSUM carries 64*h2pre) constants for the offloaded tanh
    ZS = W2SCALE * W2SCALE            # 4096
    ZP1, ZP2 = TP1 * ZS, TP2 * ZS
    ZQ1, ZQ2 = TQ1 * ZS, TQ2 * ZS
    ZCLAMP = TCLAMP * W2SCALE
    ZK = 1.0 / (15.0 * W2SCALE)       # final scale: u*r*ZK

    nb = b_core // NTILE
    n_groups = (horizon + 4) // 4
    n_blk = b_core // 128
    xcols = horizon * F
    chunks = [c for c in chunks]
    while sum(chunks) > horizon:
        chunks[-1] -= 1
        if chunks[-1] == 0:
            chunks.pop()
    if sum(chunks) < horizon:
        chunks.append(horizon - sum(chunks))
    cstart = [sum(chunks[:i]) for i in range(len(chunks))]

    nc = bacc.Bacc("TRN2", target_bir_lowering=False, debug=False,
                   num_devices=N_CORES)

    x_d = nc.dram_tensor("x", [b_core, xcols], f32, kind="ExternalInput").ap()
    w1_d = nc.dram_tensor("w1", [F, HID], f32r, kind="ExternalInput").ap()
    w2s_d = nc.dram_tensor("w2s", [128, 2 * HID], f32, kind="ExternalInput").ap()
    w3_d = nc.dram_tensor("w3dt", [HID, NST], f32r, kind="ExternalInput").ap()
    id_d = nc.dram_tensor("ident", [128, 128], f32, kind="ExternalInput").ap()
    out_d = nc.dram_tensor("out", [b_core, horizon * NST], f32,
                           kind="ExternalOutput").ap()
    hst_d = nc.dram_tensor("hstage", [NST * horizon, b_core], f32,
                           kind="Internal").ap()
    ust_d = nc.dram_tensor("ustage", [128, b_core], f32r,
                           kind="Internal").ap()

    def veng(name):
        return {"vector": nc.vector, "gpsimd": nc.gpsimd}[name]

    with tile.TileContext(nc) as tc:
        with (
            tc.tile_pool(name="persist", bufs=1) as pp,
            tc.tile_pool(name="xst", bufs=3) as xp,
            tc.tile_pool(name="sT", bufs=2) as sp_,
            tc.tile_pool(name="h1p", bufs=4) as h1p,
            tc.tile_pool(name="h2p", bufs=4) as h2p,
            tc.tile_pool(name="shb", bufs=2) as shp,
            tc.tile_pool(name="ostg", bufs=4) as op_,
            tc.tile_pool(name="tnu", bufs=2) as tnu,
            tc.tile_pool(name="tnd", bufs=2) as tnd,
            tc.tile_pool(name="tnr", bufs=2) as tnr,
            tc.tile_pool(name="psA", bufs=psa_bufs, space="PSUM") as psA,
            tc.tile_pool(name="psB", bufs=psb_bufs, space="PSUM") as psB,
            tc.tile_pool(name="psS", bufs=1, space="PSUM") as psS,
        ):
            w1sb = pp.tile([128, HID], f32r, tag="w1sb")
            w2st = pp.tile([128, 2 * HID], f32, tag="w2st")
            w2q = pp.tile([128, 2 * HID], fp8, tag="w2q")
            w3sb = pp.tile([128, 2 * NST], f32r, tag="w3sb")
            # offloaded-tanh h2 tiles carry an extra 15*W2SCALE factor; give
            # their mm3 a pre-descaled stationary instead of scaling h2
            w3off = pp.tile([128, 2 * NST], f32r, tag="w3off")
            ident = pp.tile([128, 128], f32, tag="ident")
            ut = pp.tile([128, b_core], f32r, tag="ut")  # controls.T row f*64+t

            for k in range(4):
                # per-quadrant W1, permuted to [state rows; control rows]
                nc.sync.dma_start(w1sb[32 * k:32 * k + NST, :], w1_d[NCTRL:F, :])
                nc.sync.dma_start(w1sb[32 * k + NST:32 * k + F, :],
                                  w1_d[0:NCTRL, :])
            nc.sync.dma_start(w2st[:, :], w2s_d[:, :])
            nc.vector.tensor_copy(w2q[:, :], w2st[:, :])
            nc.sync.dma_start(w3sb[:, 0:NST], w3_d[0:128, :])
            nc.sync.dma_start(w3sb[:, NST:2 * NST], w3_d[128:256, :])
            nc.sync.dma_start(ident[:, :], id_d[:, :])
            if off_tiles:
                nc.scalar.mul(w3off[:, :], w3sb[:, :], ZK)

            ust_v = ust_d.rearrange("(f t) b -> t f b", t=64)
            sgroups = [None] * (n_groups + 1)

            def fill_u(g):
                ts_ = sgroups[g]
                for s in range(4):
                    t = 4 * g + s
                    if t < horizon:
                        nc.sync.dma_start(
                            ts_[32 * s + NST:32 * s + F, :], ust_v[t])

            def alloc_group(g, fill=True):
                ts_ = sp_.tile([128, b_core], f32r, name=f"sT_g{g}", tag="sT")
                sgroups[g] = ts_
                if fill:
                    fill_u(g)
                return ts_

            for _rep in range(reps):
                st0 = alloc_group(0, fill=False)

                # ---- prologue: transpose controls (all t) and state0 ----
                bpd = min(4, n_blk)
                for q in range(n_blk // bpd):
                    rows = bpd * 128
                    cw = bpd * 128
                    xs = xp.tile([128, bpd * xcols], f32, tag="xs")
                    src = x_d[q * rows:(q + 1) * rows, :].rearrange(
                        "(j p) c -> p j c", p=128)
                    nc.sync.dma_start(
                        xs[:, :].rearrange("p (j c) -> p j c", c=xcols), src)
                    pu = psB.tile([128, 2 * NTILE], f32, tag="ph2", name="pu")
                    for fi in range(NCTRL):
                        for j in range(bpd):
                            xv = xs[:, j * xcols:(j + 1) * xcols].rearrange(
                                "p (t f) -> p t f", f=F)
                            nc.tensor.transpose(
                                pu[0:horizon,
                                   fi * NTILE + j * 128:fi * NTILE + (j + 1) * 128],
                                xv[:, :, fi], ident[:, :])
                    ps0 = psS.tile([128, NTILE], f32, tag="sm", name="ps0")
                    for j in range(bpd):
                        nc.tensor.transpose(
                            ps0[0:NST, j * 128:(j + 1) * 128],
                            xs[:, j * xcols + NCTRL:j * xcols + F],
                            ident[:, :])
                    for fi in range(NCTRL):
                        nc.vector.tensor_copy(
                            ut[64 * fi:64 * fi + horizon, q * cw:(q + 1) * cw],
                            pu[0:horizon, fi * NTILE:fi * NTILE + cw])
                    nc.vector.tensor_copy(st0[0:NST, q * cw:(q + 1) * cw],
                                          ps0[0:NST, 0:cw])
                for fi in range(NCTRL):
                    nc.sync.dma_start(ust_d[64 * fi:64 * fi + horizon, :],
                                      ut[64 * fi:64 * fi + horizon, :])
                fill_u(0)

                # ---- epilogue task queue: (chunk, blk) transposes ----
                pending = []
                shbs = {}

                BG = 4  # blocks per transpose group (BG*nrows <= 512: one PSUM bank)

                def start_chunk(k):
                    r0, nrows = cstart[k] * NST, chunks[k] * NST
                    shb = shp.tile([128, b_core], f32, tag="shb",
                                   name=f"shb{k}")
                    nc.sync.dma_start(shb[0:nrows, :],
                                      hst_d[r0:r0 + nrows, :])
                    shbs[k] = shb
                    pending.extend((k, gb) for gb in range(n_blk // BG))

                def emit_block(k, gb):
                    r0, nrows = cstart[k] * NST, chunks[k] * NST
                    shb = shbs[k]
                    ptp = {"sm": psS, "ph1": psA, "ph2": psB}[pt_tag]
                    pt = ptp.tile([128, 2 * NTILE], f32, tag=pt_tag, name="pt")
                    for i in range(BG):
                        blk = gb * BG + i
                        nc.tensor.transpose(
                            pt[0:128, i * nrows:(i + 1) * nrows],
                            shb[0:nrows, blk * 128:(blk + 1) * 128],
                            ident[0:nrows, 0:nrows])
                    ost = op_.tile([128, BG * 128], f32, tag="ost")
                    veng(epi_eng).tensor_copy(ost[:, 0:BG * nrows],
                                              pt[0:128, 0:BG * nrows])
                    dst = out_d[gb * BG * 128:(gb + 1) * BG * 128,
                                r0:r0 + nrows].rearrange(
                                    "(i p) c -> p i c", p=128)
                    nc.sync.dma_start(
                        dst, ost[:, 0:BG * nrows].rearrange(
                            "p (i c) -> p i c", c=nrows))

                # ---- main scan ----
                done_chunks = 0
                for t in range(horizon):
                    g, s = divmod(t, 4)
                    g2, s2 = divmod(t + 1, 4)
                    ts_ = sgroups[g]
                    if g2 > g:
                        alloc_group(g2)
                    stash = {}

                    def stage1(j):
                        c0, c1 = j * NTILE, (j + 1) * NTILE
                        ph1 = psA.tile([128, 2 * NTILE], f32, tag="ph1",
                                       name="ph1")
                        for m in range(2):
                            nc.tensor.matmul(
                                ph1[:, m * NTILE:(m + 1) * NTILE],
                                w1sb[32 * s:32 * s + F,
                                     m * 128:(m + 1) * 128],
                                ts_[32 * s:32 * s + F, c0:c1],
                                start=True, stop=True,
                                tile_position=(32 * s, 0))
                        h1t = h1p.tile([128, 2 * NTILE], fp8, tag="h1")
                        nc.scalar.activation(h1t[:, :], ph1[:, :], Tanh)
                        stash[j] = h1t

                    def stage2(j):
                        h1t = stash.pop(j)
                        ph2 = psB.tile([128, 2 * NTILE], f32, tag="ph2",
                                       name="ph2")
                        h1v = h1t[:, :].rearrange("p (two n) -> p two n", two=2)
                        for m in range(2):
                            nc.tensor.matmul(
                                ph2[:, m * NTILE:(m + 1) * NTILE],
                                w2q[:, m * HID:(m + 1) * HID].rearrange(
                                    "p (two m1) -> p two m1", two=2),
                                h1v, start=True, stop=True, perf_mode=DR)
                        h2t = h2p.tile([128, 2 * NTILE], f32r, tag="h2")
                        if j in off_tiles:
                            ud = tnu.tile([128, 2 * NTILE], f32, tag="u")
                            dd = tnd.tile([128, 2 * NTILE], f32, tag="d")
                            rr = tnr.tile([128, 2 * NTILE], f32, tag="r")
                            nc.vector._custom_dve(
                                tanh_den_op, out=dd[:, :], in0=ph2[:, :],
                                s0=ZQ1, s1=ZQ2, imm2=ZCLAMP)
                            nc.vector._custom_dve(
                                tanh_num_op, out=ud[:, :], in0=ph2[:, :],
                                s0=ZP1, s1=ZP2, imm2=ZCLAMP)
                            nc.vector.reciprocal_approx_fast(rr[:, :], dd[:, :])
                            # h2t = u*r = 960*tanh; stage3 compensates via w3off
                            veng(mul_eng).tensor_tensor(
                                h2t[:, :], ud[:, :], rr[:, :], Alu.mult)
                        else:
                            nc.scalar.activation(h2t[:, :], ph2[:, :], Tanh,
                                                 scale=1.0 / W2SCALE)
                        stash[("h2", j)] = h2t

                    def stage3(j):
                        h2t = stash.pop(("h2", j))
                        if j % 2 == 0:
                            if pd_tag == "ph2":
                                pdt = psB.tile([128, 2 * NTILE], f32,
                                               tag="ph2", name="pd")
                            elif pd_tag == "ph1":
                                pdt = psA.tile([128, 2 * NTILE], f32,
                                               tag="ph1", name="pd")
                            else:
                                pdt = psS.tile([128, 2 * NTILE], f32,
                                               tag="sm", name="pd")
                            stash["pd"] = pdt
                        pdt = stash["pd"]
                        dcol = (j % 2) * NTILE
                        w3u = w3off if j in off_tiles else w3sb
                        nc.tensor.matmul(pdt[0:NST, dcol:dcol + NTILE],
                                         w3u[:, 0:NST], h2t[:, 0:NTILE],
                                         start=True, stop=False)
                        nc.tensor.matmul(pdt[0:NST, dcol:dcol + NTILE],
                                         w3u[:, NST:2 * NST],
                                         h2t[:, NTILE:2 * NTILE],
                                         start=False, stop=True)
                        if j < 2:
                            p0, pw = j * NTILE, NTILE
                        elif j % 2 == 1 or j == nb - 1:
                            p0 = (j - j % 2) * NTILE
                            pw = (j % 2 + 1) * NTILE
                        else:
                            p0 = None
                        if p0 is not None:
                            # state(t+1) = state(t) + d (W3 pre-scaled by DT)
                            veng(state_eng).scalar_tensor_tensor(
                                sgroups[g2][32 * s2:32 * s2 + NST, p0:p0 + pw],
                                pdt[0:NST, (p0 % (2 * NTILE)):
                                    (p0 % (2 * NTILE)) + pw],
                                1.0,
                                ts_[32 * s:32 * s + NST, p0:p0 + pw],
                                Alu.mult, Alu.add)

                    for j in range(nb):
                        stage1(j)
                        if j >= 1:
                            stage2(j - 1)
                            stage3(j - 1)
                    stage2(nb - 1)
                    stage3(nb - 1)

                    # record state(t+1) as output row t (DRAM staging)
                    nc.sync.dma_start(
                        hst_d[NST * t:NST * (t + 1), :],
                        sgroups[g2][32 * s2:32 * s2 + NST, :].bitcast(f32))

                    # interleave output transposes for completed chunks
                    if (done_chunks < len(chunks)
                            and t + 1 == cstart[done_chunks] + chunks[done_chunks]):
                        start_chunk(done_chunks)
                        done_chunks += 1
                    for _ in range(min(spread, len(pending))):
                        emit_block(*pending.pop(0))

                while done_chunks < len(chunks):
                    start_chunk(done_chunks)
                    done_chunks += 1
                while pending:
                    emit_block(*pending.pop(0))

    nc.compile()
    return nc


def _get_nc(b_core=B_CORE, horizon=H, **kw):
    key = (b_core, horizon, tuple(sorted(kw.items())))
    if key not in _CACHE:
        _CACHE[key] = _build(b_core, horizon, **kw)
    return _CACHE[key]


def _run(x, W1, b1, W2, b2, W3, b3, nc_kwargs=None, **spmd_kwargs):
    import concourse.bass_utils as bass_utils

    x = np.ascontiguousarray(np.asarray(x, dtype=np.float32))
    W1 = np.ascontiguousarray(np.asarray(W1, dtype=np.float32))
    W2 = np.ascontiguousarray(np.asarray(W2, dtype=np.float32))
    W3 = np.ascontiguousarray(np.asarray(W3, dtype=np.float32))
    for b in (b1, b2, b3):
        assert not np.any(np.asarray(b)), "kernel built for zero biases"

    nc = _get_nc(**(nc_kwargs or {}))
    w3dt = np.ascontiguousarray(DT * W3)
    # W2 pre-scaled and rearranged for fp8 DoubleRow stationary layout:
    # w2s[p, m*256 + i*128 + m'] = 64*W2[p + 128*i, 128*m + m']
    w2s = (W2SCALE * W2).reshape(2, 128, 2, 128).transpose(1, 2, 0, 3)
    w2s = np.ascontiguousarray(w2s.reshape(128, 2 * HID), dtype=np.float32)
    ident = np.eye(128, dtype=np.float32)
    xr = x.reshape(B_TOTAL, H * F)

    in_maps = []
    for c in range(N_CORES):
        in_maps.append({
            "x": xr[c * B_CORE:(c + 1) * B_CORE],
            "w1": W1, "w2s": w2s, "w3dt": w3dt, "ident": ident,
        })
    res = bass_utils.run_bass_kernel_spmd(nc, in_maps,
                                          core_ids=list(range(N_CORES)),
                                          **spmd_kwargs)
    out = np.concatenate(
        [res.results[c]["out"].reshape(B_CORE, H, NST) for c in range(N_CORES)],
        axis=0)
    return out, res


def kernel(x, W1, b1, W2, b2, W3, b3):
    out, _ = _run(x, W1, b1, W2, b2, W3, b3)
    return out
